# revision 31
# baseline (speedup 1.0000x reference)
"""Trainium2 Bass kernel for causal (strict-future-masked) MHA + residual + LayerNorm.

Reference semantics (Keras MultiHeadAttention, inference):
    q,k,v = einsum(x, W{q,k,v})        [B,S,H,DH]  (biases are zero per spec)
    scores = q.k / sqrt(DH); mask allows j > i (STRICT UPPER triangle);
    ctx = softmax(scores) @ v; out = ctx @ Wo; y = LN(x + out).

Shapes: B=2, S=2048, D=1024, H=16, DH=64.

Sharding (8 cores): core c -> batch b = c//4, head-group hg = c%4 (4 heads),
RS rank r = c%4. Each core computes q/k/v + attention + out-proj partial for
its 4 heads over the full sequence, ReduceScatter([2048,1024] bf16) within
its 4-core batch group yields rows [512r, 512r+512) of the head-summed
attn_out, then residual + LayerNorm locally. Host assembles 8 x [512,1024].

Precision/engine plan (cost-model driven):
  x, Wq/Wk/Wv/Wo in fp8e4m3, pair-packed for DoubleRow matmuls (0.5 PE
  cycles/row, 2x contraction per instruction = 4x fp32r throughput).
  q,k stored bf16; score pairs accumulate f32 into 2-bank PSUM tiles; ONE
  exp per kb-pair ([128,1024] on the Act engine - the ~70us bottleneck this
  schedule is built around) with fp8 output and bias -1.5 (keeps E inside
  fp8e4m3 range; cancels in softmax). Causal masks pre-exp as additive
  -1e9 accumulated ON THE PE (ident.T @ mask matmuls). E@v and out-proj in
  fp8 DoubleRow. Softmax: DVE reciprocal of the ones-column Z row + Pool
  partition_broadcast + DVE multiply -> fp8 ctx. Out-proj stages bf16 via
  DVE; chunked bf16 ReduceScatter per q-block overlaps the next block's
  attention; LN fully on DVE (rsqrt = bit-trick + 2 Newton steps, so the
  Act engine only ever runs Copy/Exp - one act-table load), deferred ~2
  groups for overlap. Emission is software-pipelined: each pair's sT/exp
  is emitted before the previous pair's E@v so Act never starves, and all
  bulk DMAs are coalesced (4 x-tiles, 1 weight, 1 mask) because DMA issue
  costs ~1.1us each on a sequencer.
  Measured numerics (numpy emulation of this pipeline): rel ~7.5e-3.
"""

import numpy as np
import ml_dtypes

B, S, D, H, DH = 2, 2048, 1024, 16, 64
HPC = 4            # heads per core
NCORES = 8
QB = 512           # q-block
NQB = S // QB      # 4
KBLK = 128         # kpos block
NKB = S // KBLK    # 16
NEG = -1.0e9
SCALE = 1.0 / 8.0  # 1/sqrt(DH)
EBIAS = -1.5       # exp bias: cancels in softmax, keeps E in fp8 range
EPS = 1.0e-6
MAGIC = 0x5F3759DF  # rsqrt seed

NP_FP8 = ml_dtypes.float8_e4m3
NP_BF16 = ml_dtypes.bfloat16

_CACHE = {}


def _build_program(with_collective=True, ln_affine=False):
    """Build + compile the SPMD Bass program (identical on all 8 cores)."""
    import concourse.bass as bass
    import concourse.tile as tile
    from concourse import bacc, mybir

    f32 = mybir.dt.float32
    f32r = mybir.dt.float32r
    i32 = mybir.dt.int32
    bf16 = mybir.dt.bfloat16
    fp8 = mybir.dt.float8e4
    DR = mybir.MatmulPerfMode.DoubleRow
    Alu = mybir.AluOpType
    Act = mybir.ActivationFunctionType

    nc = bacc.Bacc("TRN2", target_bir_lowering=False, debug=False,
                   num_devices=NCORES)

    # ---- external I/O (coalesced: few big DMAs, issue costs ~1.1us each) --
    xt8_d = nc.dram_tensor("xt8", [512, 2 * S], fp8, kind="ExternalInput").ap()
    # w8 = [wq(512) | wk(512) | wv(520)] per pair-row
    w8_d = nc.dram_tensor("w8", [512, 1544], fp8, kind="ExternalInput").ap()
    wo8_d = nc.dram_tensor("wo8", [128, 2048], fp8, kind="ExternalInput").ap()
    # masks = [band(128) | band3(128) | r0(256) | ident(128)]
    mask_d = nc.dram_tensor("mask8", [128, 640], bf16,
                            kind="ExternalInput").ap()
    xres = nc.dram_tensor("xres", [QB, D], f32, kind="ExternalInput").ap()
    if ln_affine:
        gamma_r = nc.dram_tensor("gamma_r", [1, D], f32,
                                 kind="ExternalInput").ap()
        beta_r = nc.dram_tensor("beta_r", [1, D], f32,
                                kind="ExternalInput").ap()
    out = nc.dram_tensor("out", [QB, D], f32, kind="ExternalOutput").ap()

    attn_dram_l = [nc.dram_tensor(f"attn_dram{j}", [QB, D], bf16)
                   for j in range(NQB)]
    rs_dram_l = [nc.dram_tensor(f"rs_dram{j}", [128, D], bf16)
                 for j in range(NQB)]

    with tile.TileContext(nc) as tc:
        from contextlib import ExitStack
        with ExitStack() as ctx:
            # ---------- persistent pools ----------
            p_rows = ctx.enter_context(tc.tile_pool(name="rows", bufs=1))
            p_w = ctx.enter_context(tc.tile_pool(name="w", bufs=1))
            p_xt = ctx.enter_context(tc.tile_pool(name="xt", bufs=1))
            p_qk = ctx.enter_context(tc.tile_pool(name="qk", bufs=1))
            p_v = ctx.enter_context(tc.tile_pool(name="v", bufs=1))
            p_ctx = ctx.enter_context(tc.tile_pool(name="ctxp", bufs=1))
            p_bc = ctx.enter_context(tc.tile_pool(name="bc", bufs=1))
            p_e = ctx.enter_context(tc.tile_pool(name="e", bufs=6))
            p_z = ctx.enter_context(tc.tile_pool(name="zrow", bufs=4))
            p_bcn = ctx.enter_context(tc.tile_pool(name="bcn", bufs=3))
            p_ln = ctx.enter_context(tc.tile_pool(name="ln", bufs=2))
            p_lnst = ctx.enter_context(tc.tile_pool(name="lnst", bufs=2))
            p_stage = ctx.enter_context(tc.tile_pool(name="stage", bufs=2))

            # small constants
            ebias_col = p_rows.tile([128, 1], f32, name="ebias_col",
                                    tag="ebias_col")
            nc.vector.memset(ebias_col[:], EBIAS)
            ones8 = p_rows.tile([128, 2, 1], fp8, name="ones8", tag="ones8")
            nc.vector.memset(ones8[:], 1.0)
            one_f32 = p_rows.tile([1, 1], f32, name="one_f32", tag="one_f32")
            nc.vector.memset(one_f32[:], 1.0)
            ones64 = p_rows.tile([1, 64], f32, name="ones64", tag="ones64")
            nc.vector.memset(ones64[:], 1.0)
            sv_row = p_rows.tile([1, 260], f32, name="sv_row", tag="sv_row")

            # bulk loads (SP queue: 3 issues; Pool queue: 4 xt issues)
            w_all = p_w.tile([128, 4, 1544], fp8, name="w_all", tag="w_all")
            w8_r = w8_d.rearrange("(p r) c -> r p c", p=4)
            nc.sync.dma_start(w_all[:, :, 512:1024], w8_r[:, :, 512:1024])
            nc.sync.dma_start(w_all[:, :, 0:512], w8_r[:, :, 0:512])
            nc.sync.dma_start(w_all[:, :, 1024:1544], w8_r[:, :, 1024:1544])
            m_all = p_w.tile([128, 640], bf16, name="m_all", tag="m_all")
            nc.sync.dma_start(m_all[:], mask_d[:])
            wo_sb = p_w.tile([128, 2, 1024], fp8, name="wo8", tag="wo8")
            band_add = m_all[:, 0:128]
            band3_add = m_all[:, 128:256]
            r0_add = m_all[:, 256:512]
            ident = m_all[:, 512:640]
            if ln_affine:
                gamma_row = p_rows.tile([1, D], f32, name="gamma_row",
                                        tag="gamma_row")
                nc.sync.dma_start(gamma_row[:], gamma_r[:])
                beta_row = p_rows.tile([1, D], f32, name="beta_row",
                                       tag="beta_row")
                nc.sync.dma_start(beta_row[:], beta_r[:])
                gamma_bc = p_bc.tile([128, D], f32, name="gamma_bc",
                                     tag="gamma_bc")
                nc.gpsimd.partition_broadcast(gamma_bc[:], gamma_row[:])
                beta_bc = p_bc.tile([128, D], f32, name="beta_bc",
                                    tag="beta_bc")
                nc.gpsimd.partition_broadcast(beta_bc[:], beta_row[:])

            # x tiles: 2 DMAs per pair-row; token cols [1024:2048] first
            # (attention starts at qb=3 and QKV runs nbp=1 jobs first)
            xt_t = []
            for p in range(4):
                t = p_xt.tile([128, 2, S], fp8, name=f"xt{p}", tag=f"xt{p}")
                xt_t.append(t)
            rows = [xt8_d[128 * p:128 * p + 128, :].rearrange(
                "p (l c) -> p l c", c=S) for p in range(4)]
            for hv in (1, 0):
                for p in range(4):
                    nc.gpsimd.dma_start(
                        xt_t[p][:, :, 1024 * hv:1024 * hv + 1024],
                        rows[p][:, :, 1024 * hv:1024 * hv + 1024])

            # wo load on the Pool queue, behind the xt DMAs
            nc.gpsimd.dma_start(wo_sb.rearrange("p a b -> p (a b)"), wo8_d[:])
            # xres chunks preloaded behind the weights (no deps; Pool queue)
            xres_t = []
            for j in range(NQB):
                xrj = p_ln.tile([128, D], f32, name=f"xr{j}", tag=f"xr{j}",
                                bufs=1)
                nc.gpsimd.dma_start(xrj[:],
                                    xres[128 * j:128 * j + 128, :])
                xres_t.append(xrj)

            def wq_ap(p, t2):
                return w_all[:, p, 0:512].rearrange(
                    "p (a b) -> p a b", b=256)[:, :, 128 * t2:128 * t2 + 128]

            def wk_ap(p, t2):
                return w_all[:, p, 512:1024].rearrange(
                    "p (a b) -> p a b", b=256)[:, :, 128 * t2:128 * t2 + 128]

            def wv_ap(p):
                return w_all[:, p, 1024:1544].rearrange(
                    "p (a b) -> p a b", b=260)

            # persistent activations
            qT_sb = [p_qk.tile([128, S], bf16, name=f"qT{t2}", tag=f"qT{t2}")
                     for t2 in range(2)]
            kT_sb = [p_qk.tile([128, S], bf16, name=f"kT{t2}", tag=f"kT{t2}")
                     for t2 in range(2)]
            # v: [kpos, kb, 130*t2 + 65*half + dh]; col 65j+64 is the ones col
            v_sb = p_v.tile([128, NKB, 260], fp8, name="v_sb", tag="v_sb")
            ctx_sb = p_ctx.tile([128, 2, S], fp8, name="ctxs", tag="ctxs")

            # ---------- phase 1: QKV projections (fp8 DoubleRow) ----------
            with tc.tile_pool(name="ps_qkv", bufs=2, space="PSUM") as ps_qkv:
                # qk accs rotate 3-deep so PE isn't paced by the copies

                # interleave q/k accs (Act copies) with v accs (DVE copies)
                # k before q, nbp=1 first: attention starts with qb=3,
                # which needs only the nbp=1 halves of qT/kT and v[12:16]
                qk_jobs = [(d, t2, nbp) for nbp in (1, 0)
                           for d in (1, 0) for t2 in range(2)]
                v_jobs = list(range(NKB - 1, 7, -1))
                vi = 0
                # ones columns first; v copies skip them (strided)
                vcols = v_sb.rearrange("p k (j c) -> p k j c", c=65)
                nc.vector.memset(vcols[:, :, :, 64:65], 1.0)

                def emit_v(tb):
                    win, tsub = tb // 4, tb % 4
                    accv = ps_qkv.tile([128, 260], f32, name="vp",
                                       tag="vp", bufs=2)
                    for p in range(4):
                        nc.tensor.matmul(
                            accv[:],
                            xt_t[p][:, :, 512 * win + 128 * tsub:
                                    512 * win + 128 * tsub + 128],
                            wv_ap(p), start=(p == 0), stop=(p == 3),
                            perf_mode=DR)
                    nc.vector.tensor_copy(
                        vcols[:, tb, :, 0:64],
                        accv.rearrange("p (j c) -> p j c", c=65)[:, :, 0:64])

                for ji, (d, t2, nbp) in enumerate(qk_jobs):
                    w_ap = wq_ap if d == 0 else wk_ap
                    dst = qT_sb[t2] if d == 0 else kT_sb[t2]
                    acc = ps_qkv.tile([128, 1024], f32, name="qkp",
                                      tag="qkp", bufs=3)
                    for w2 in (1, 0):
                        nb = 2 * nbp + w2
                        for p in range(4):
                            nc.tensor.matmul(
                                acc[:, 512 * w2:512 * w2 + 512],
                                w_ap(p, t2),
                                xt_t[p][:, :, 512 * nb:512 * nb + 512],
                                start=(p == 0), stop=(p == 3),
                                perf_mode=DR, skip_group_check=True)
                        if ji < 2:  # first k copies: high window first
                            nc.scalar.copy(
                                dst[:, 1024 * nbp + 512 * w2:
                                    1024 * nbp + 512 * w2 + 512],
                                acc[:, 512 * w2:512 * w2 + 512])
                    if ji >= 2:
                        if ji % 2 == 0:
                            nc.vector.tensor_copy(
                                dst[:, 1024 * nbp:1024 * nbp + 1024], acc[:])
                        else:
                            nc.scalar.copy(
                                dst[:, 1024 * nbp:1024 * nbp + 1024], acc[:])
                    if vi < len(v_jobs):  # 1 v acc per qk acc
                        emit_v(v_jobs[vi])
                        vi += 1
                while vi < len(v_jobs):
                    emit_v(v_jobs[vi])
                    vi += 1

            # phase 2 PSUM pools
            ps_s = ctx.enter_context(
                tc.tile_pool(name="ps_s", bufs=2, space="PSUM"))
            ps_c = ctx.enter_context(
                tc.tile_pool(name="ps_c", bufs=2, space="PSUM"))
            ps_o = ctx.enter_context(
                tc.tile_pool(name="ps_o", bufs=1, space="PSUM"))

            # ---------- LN (all-DVE rsqrt; no act-table switches) ----------
            def emit_ln(j, tail=False):
                xr = xres_t[j]
                y8 = p_ln.tile([128, D], bf16, name="y8", tag="y8")
                nc.gpsimd.dma_start(y8[:], rs_dram_l[j][:])
                y = p_ln.tile([128, D], f32, name="y", tag="y")
                ysum = p_lnst.tile([128, 1], f32, name="ysum", tag="ysum")
                nc.vector.scalar_tensor_tensor(
                    y[:], y8[:], 1.0, xr[:], Alu.mult, Alu.add,
                    accum_out=ysum[:])
                negmu = p_lnst.tile([128, 1], f32, name="negmu", tag="negmu")
                nc.vector.tensor_scalar_mul(negmu[:], ysum[:],
                                            -1.0 / float(D))
                ysq = p_ln.tile([128, D], f32, name="ysq", tag="ysq")
                ssum = p_lnst.tile([128, 1], f32, name="ssum", tag="ssum")
                nc.vector.scalar_tensor_tensor(ysq[:], y[:], 0.0, y[:],
                                               Alu.add, Alu.mult,
                                               accum_out=ssum[:])
                # var = ssum/D + eps - mu^2
                mu2 = p_lnst.tile([128, 1], f32, name="mu2", tag="mu2")
                nc.vector.tensor_tensor(mu2[:], negmu[:], negmu[:], Alu.mult)
                var = p_lnst.tile([128, 1], f32, name="var", tag="var")
                nc.vector.tensor_scalar(var[:], ssum[:], 1.0 / float(D), EPS,
                                        Alu.mult, Alu.add)
                nc.vector.tensor_tensor(var[:], var[:], mu2[:], Alu.subtract)
                # rstd = rsqrt(var): bit-trick seed + 2 Newton iterations
                r = p_lnst.tile([128, 1], f32, name="r", tag="r")
                nc.vector.tensor_scalar(r.bitcast(i32)[:],
                                        var.bitcast(i32)[:],
                                        1, -1, Alu.logical_shift_right,
                                        Alu.mult)
                nc.vector.tensor_scalar_add(r.bitcast(i32)[:],
                                            r.bitcast(i32)[:], MAGIC)
                t_ = p_lnst.tile([128, 1], f32, name="t_", tag="t_")
                for _ in range(2):
                    nc.vector.tensor_tensor(t_[:], r[:], r[:], Alu.mult)
                    nc.vector.tensor_tensor(t_[:], t_[:], var[:], Alu.mult)
                    nc.vector.tensor_scalar(t_[:], t_[:], -0.5, 1.5,
                                            Alu.mult, Alu.add)
                    nc.vector.tensor_tensor(r[:], r[:], t_[:], Alu.mult)
                if ln_affine:
                    yc = p_ln.tile([128, D], f32, name="yc", tag="ysq")
                    nc.vector.tensor_scalar(yc[:], y[:], negmu[:], r[:],
                                            Alu.add, Alu.mult)
                    nc.vector.scalar_tensor_tensor(
                        y[:], yc[:], 1.0, gamma_bc[:], Alu.mult, Alu.mult)
                    nc.vector.tensor_add(y[:], y[:], beta_bc[:])
                    nc.sync.dma_start(out[128 * j:128 * j + 128, :], y[:])
                elif tail:
                    # split final scale into halves to overlap the out DMA
                    for hv in range(2):
                        sl = slice(512 * hv, 512 * hv + 512)
                        nc.vector.tensor_scalar(ysq[:, sl], y[:, sl],
                                                negmu[:], r[:],
                                                Alu.add, Alu.mult)
                        nc.sync.dma_start(out[128 * j:128 * j + 128, sl],
                                          ysq[:, sl])
                else:
                    nc.vector.tensor_scalar(ysq[:], y[:], negmu[:], r[:],
                                            Alu.add, Alu.mult)
                    nc.sync.dma_start(out[128 * j:128 * j + 128, :], ysq[:])

            def emit_finalize(qb, tail=False):
                """sv-fix (qb==3), out-proj + chunked RS for qb."""
                if qb == NQB - 1:
                    # q = S-1 is fully masked: ctx col <- mean(v).
                    # one [1,2,64] lhsT per t2 maps sv cols onto the 128
                    # ctx partitions (64*half + dh) in a single matmul.
                    for t2 in range(2):
                        svl = sv_row[:, 130 * t2:130 * t2 + 130].rearrange(
                            "a (h c) -> a h c", c=65)[:, :, 0:64]
                        svc = ps_o.tile([128, 1], f32, name="svc", tag="op")
                        nc.tensor.matmul(
                            svc[:], svl, one_f32[:], start=True, stop=True,
                            skip_group_check=True)
                        nc.vector.tensor_scalar_mul(
                            ctx_sb[:, t2, S - 1:S], svc[:], 1.0 / float(S))
                for qtl in range(4):
                    qt = 4 * qb + qtl
                    pool = ps_s if tail else ps_o  # ps_s idle at tail
                    acc = pool.tile([128, 1024], f32, name="op",
                                    tag="sT" if tail else "op")
                    for ob in range(2):
                        nc.tensor.matmul(
                            acc[:, 512 * ob:512 * ob + 512],
                            ctx_sb[:, :, 128 * qt:128 * qt + 128],
                            wo_sb[:, :, 512 * ob:512 * ob + 512],
                            start=True, stop=True, perf_mode=DR,
                            skip_group_check=True)
                    stage = p_stage.tile([128, 1024], bf16, name="stage",
                                         tag="stage")
                    if tail and qtl % 2 == 1:
                        nc.scalar.copy(stage[:], acc[:])  # Act free at tail
                    else:
                        nc.vector.tensor_copy(stage[:], acc[:])
                    nc.sync.dma_start(
                        attn_dram_l[qb][128 * qtl:128 * qtl + 128, :],
                        stage[:])
                    if qtl == 0 and not with_collective:
                        # timing variant: rank-0 copy only needs qt0's rows
                        nc.sync.dma_start(rs_dram_l[qb][:],
                                          attn_dram_l[qb][0:128, :])
                if with_collective:
                    nc.gpsimd.collective_compute(
                        "ReduceScatter",
                        Alu.add,
                        replica_groups=[[0, 1, 2, 3], [4, 5, 6, 7]],
                        ins=[attn_dram_l[qb][:]],
                        outs=[rs_dram_l[qb][:]],
                    )

            # ---------- phase 2: attention, software-pipelined ----------
            # low-half v projections (kb 7..0) + mean(v) ride one-per-step
            # inside qb=3's exp stream, using the idle out-proj PSUM slot
            def mk_inject_v(tb):
                def run():
                    accv = ps_o.tile([128, 260], f32, name="vpi", tag="op")
                    win, tsub = tb // 4, tb % 4
                    for p in range(4):
                        nc.tensor.matmul(
                            accv[:],
                            xt_t[p][:, :, 512 * win + 128 * tsub:
                                    512 * win + 128 * tsub + 128],
                            wv_ap(p), start=(p == 0), stop=(p == 3),
                            perf_mode=DR)
                    nc.vector.tensor_copy(
                        vcols[:, tb, :, 0:64],
                        accv.rearrange("p (j c) -> p j c", c=65)[:, :, 0:64])
                return run

            def mk_sv():
                def run():
                    svp = ps_o.tile([1, 260], f32, name="svp", tag="op")
                    for kbp in range(8):
                        nc.tensor.matmul(
                            svp[:], ones8[:],
                            v_sb[:, 2 * kbp:2 * kbp + 2, :],
                            start=(kbp == 0), stop=(kbp == 7),
                            perf_mode=DR, skip_group_check=True)
                    nc.vector.tensor_copy(sv_row[:], svp[:])
                return run

            inject_q = [mk_inject_v(tb) for tb in range(7, -1, -1)]
            inject_q.append(mk_sv())

            backlog = []

            def step(front, back):
                front()
                if inject_q:
                    inject_q.pop(0)()
                if len(backlog) >= 2:
                    backlog.pop(0)()
                backlog.append(back)

            # extras chained after a given (qb, group)'s last E@v.
            # qb order is 3,2,1,0: the short qb=3 block leads, so each
            # finalize/LN rides inside the next (longer) qb's exp stream.
            extras = {
                (2, 0): [lambda: emit_finalize(3)],
                (2, 2): [lambda: emit_ln(3)],
                (2, 3): [lambda: emit_finalize(2)],
                (1, 1): [lambda: emit_ln(2)],
                (1, 3): [lambda: emit_finalize(1)],
                (0, 1): [lambda: emit_ln(1)],
            }

            for qb in (3, 2, 1, 0):
                for g in range(4):
                    t2, half = g // 2, g % 2
                    po = 64 * half
                    ctxu = ps_c.tile([65, QB], f32, name="ctxu", tag="ctxu")
                    kT = kT_sb[t2]
                    qT = qT_sb[t2]
                    qc = QB * qb

                    def mk_full(kb0, first, ctxu=ctxu, kT=kT, qT=qT,
                                qc=qc, po=po, t2=t2, half=half, qb=qb):
                        diag = (kb0 == 4 * qb + 2)
                        cell = []

                        def front():
                            sT = ps_s.tile([128, 1024], f32, name="sT",
                                           tag="sT")
                            for lane in range(2):
                                nc.tensor.matmul(
                                    sT[:, 512 * lane:512 * lane + 512],
                                    kT[po:po + 64,
                                       128 * (kb0 + lane):
                                       128 * (kb0 + lane) + 128],
                                    qT[po:po + 64, qc:qc + 512],
                                    start=True, stop=not diag,
                                    skip_group_check=True)
                            if diag:
                                nc.tensor.matmul(
                                    sT[:, 256:512], ident, r0_add,
                                    start=False, stop=True,
                                    skip_group_check=True)
                                bnd = band3_add if qb == NQB - 1 \
                                    else band_add
                                nc.tensor.matmul(
                                    sT[:, 896:1024], ident, bnd,
                                    start=False, stop=True,
                                    skip_group_check=True)
                            e = p_e.tile([128, 2, 512], fp8, name="e_t",
                                         tag="e_t")
                            nc.scalar.activation(
                                e.rearrange("p a b -> p (a b)"), sT[:],
                                Act.Exp, scale=SCALE, bias=ebias_col[:])
                            cell.append(e)

                        def back(first=first):
                            nc.tensor.matmul(
                                ctxu[:],
                                v_sb[:, kb0:kb0 + 2,
                                     130 * t2 + 65 * half:
                                     130 * t2 + 65 * half + 65],
                                cell[0][:], start=first, stop=False,
                                perf_mode=DR, skip_group_check=True)
                        return front, back

                    def mk_diag_lo(ctxu=ctxu, kT=kT, qT=qT, qc=qc,
                                   po=po, t2=t2, half=half, qb=qb):
                        kb0 = 4 * qb
                        cell = []

                        def front():
                            sT = ps_s.tile([128, 1024], f32, name="sT",
                                           tag="sT")
                            # one bank, one group: s0(T,F) m0 s1 m1(F,T)
                            nc.tensor.matmul(
                                sT[:, 0:256],
                                kT[po:po + 64, 128 * kb0:128 * kb0 + 128],
                                qT[po:po + 64, qc:qc + 256],
                                start=True, stop=False,
                                skip_group_check=True)
                            nc.tensor.matmul(
                                sT[:, 0:256], ident, r0_add,
                                start=False, stop=False,
                                skip_group_check=True)
                            nc.tensor.matmul(
                                sT[:, 256:512],
                                kT[po:po + 64,
                                   128 * kb0 + 128:128 * kb0 + 256],
                                qT[po:po + 64, qc:qc + 256],
                                start=False, stop=False,
                                skip_group_check=True)
                            nc.tensor.matmul(
                                sT[:, 384:512], ident, band_add,
                                start=False, stop=True,
                                skip_group_check=True)
                            e = p_e.tile([128, 2, 512], fp8, name="e_t",
                                         tag="e_t")
                            nc.scalar.activation(
                                e[:, :, 0:256],
                                sT[:, 0:512].rearrange(
                                    "p (a b) -> p a b", b=256),
                                Act.Exp, scale=SCALE, bias=ebias_col[:])
                            cell.append(e)

                        def back():
                            nc.tensor.matmul(
                                ctxu[:, 0:256],
                                v_sb[:, kb0:kb0 + 2,
                                     130 * t2 + 65 * half:
                                     130 * t2 + 65 * half + 65],
                                cell[0][:, :, 0:256], start=False, stop=True,
                                perf_mode=DR, skip_group_check=True)
                        return front, back

                    def mk_norm(ctxu=ctxu, po=po, t2=t2, qc=qc):
                        def norm():
                            zinv = p_z.tile([1, QB], f32, name="zinv",
                                            tag="zinv")
                            nc.vector.reciprocal(zinv[:], ctxu[64:65, :])
                            zbs = p_bcn.tile([64, QB], f32, name="zbs",
                                             tag="zbs")
                            nc.gpsimd.partition_broadcast(zbs[:], zinv[:])
                            nc.vector.tensor_tensor(
                                ctx_sb[po:po + 64, t2, qc:qc + QB],
                                ctxu[0:64, :], zbs[:], Alu.mult)
                        return norm

                    pairs = []
                    first = True
                    for kbp in range(7, 2 * qb + 1, -1):  # full pairs
                        pairs.append(mk_full(2 * kbp, first))
                        first = False
                    pairs.append(mk_full(4 * qb + 2, first))  # diag-hi
                    pairs.append(mk_diag_lo())                # diag-lo

                    ext = extras.get((qb, g), [])
                    for i, (front, back) in enumerate(pairs):
                        if i == len(pairs) - 1:
                            norm = mk_norm()

                            def fin(b=back, n=norm, ex=tuple(ext)):
                                b()
                                n()
                                for e_ in ex:
                                    e_()
                            step(front, fin)
                        else:
                            step(front, back)

            while backlog:
                backlog.pop(0)()
            emit_finalize(0, tail=True)
            emit_ln(0, tail=True)

    nc.compile()
    return nc


def _get_program(with_collective=True, ln_affine=False):
    key = ("prog", with_collective, ln_affine)
    if key not in _CACHE:
        _CACHE[key] = _build_program(with_collective, ln_affine)
    return _CACHE[key]


def _pack_pairs(a):
    """[1024, C] -> [512, 2C]: rows 128p+r, cols C*l+c = a[256p+128l+r, c]."""
    C = a.shape[1]
    o = np.empty((512, 2 * C), a.dtype)
    for p in range(4):
        o[128 * p:128 * p + 128, 0:C] = a[256 * p:256 * p + 128]
        o[128 * p:128 * p + 128, C:2 * C] = a[256 * p + 128:256 * p + 256]
    return o


def _host_prep(x, Wq, bq, Wk, bk, Wv, bv, Wo, bo, gamma, beta):
    """Build the 8 per-core input dicts."""
    x = np.ascontiguousarray(np.asarray(x, np.float32))
    WqR = np.asarray(Wq, np.float32).reshape(D, H * DH)
    WkR = np.asarray(Wk, np.float32).reshape(D, H * DH)
    WvR = np.asarray(Wv, np.float32).reshape(D, H * DH)
    WoR = np.asarray(Wo, np.float32).reshape(H * DH, D)
    boF = np.asarray(bo, np.float32).reshape(D)
    gF = np.asarray(gamma, np.float32).reshape(D)
    btF = np.asarray(beta, np.float32).reshape(D)

    xT8 = [np.ascontiguousarray(_pack_pairs(x[b].T.astype(NP_FP8)))
           for b in range(B)]

    # additive causal masks (bf16, applied via ident.T @ mask on the PE):
    # within diagonal block kb = 4*qb+rr, (i, jj) allowed iff 128*rr+i > jj.
    # band3: qb=3 guard - col 127 (q = S-1) left open so Z stays positive
    # (that ctx column is overwritten with mean(v)/S afterwards).
    i = np.arange(128)[:, None]
    jj = np.arange(128)[None, :]
    band_add = np.where(i > jj, 0.0, NEG).astype(NP_BF16)
    band3_add = band_add.copy()
    band3_add[:, 127] = 0.0
    r0_add = np.concatenate(
        [band_add, np.full((128, 128), NEG, NP_BF16)], axis=1)
    ident = np.eye(128, dtype=NP_BF16)
    mask8 = np.concatenate([band_add, band3_add, r0_add, ident], axis=1)

    in_maps = []
    for c in range(NCORES):
        b, hg = c // 4, c % 4
        cols = slice(256 * hg, 256 * hg + 256)
        wv_c = np.zeros((D, 260), np.float32)
        for t2 in range(2):
            for j2 in range(2):
                h2 = 2 * t2 + j2
                wv_c[:, 130 * t2 + 65 * j2:130 * t2 + 65 * j2 + 64] = \
                    WvR[:, 256 * hg + 64 * h2:256 * hg + 64 * h2 + 64]
        wo_c = WoR[cols, :].astype(NP_FP8)  # [256, 1024]
        wo8 = np.empty((128, 2048), NP_FP8)
        wo8[:, 0:1024] = wo_c[0:128]
        wo8[:, 1024:2048] = wo_c[128:256]
        w8 = np.concatenate([
            _pack_pairs(WqR[:, cols].astype(NP_FP8)),
            _pack_pairs(WkR[:, cols].astype(NP_FP8)),
            _pack_pairs(wv_c.astype(NP_FP8)),
        ], axis=1)
        in_maps.append({
            "xt8": xT8[b],
            "w8": w8,
            "wo8": wo8,
            "mask8": mask8,
            "xres": boF[None, :] + np.concatenate(
                [x[b, QB * j + 128 * hg:QB * j + 128 * hg + 128]
                 for j in range(NQB)], axis=0),
            "gamma_r": gF[None, :].copy(),
            "beta_r": btF[None, :].copy(),
        })
    return in_maps


def kernel(**inputs):
    from concourse.bass_utils import run_bass_kernel_spmd

    gamma = np.asarray(inputs["gamma"], np.float32)
    beta = np.asarray(inputs["beta"], np.float32)
    ln_affine = not (np.all(gamma == 1.0) and np.all(beta == 0.0))
    nc = _get_program(with_collective=True, ln_affine=ln_affine)
    in_maps = _host_prep(**inputs)
    if not ln_affine:
        for m in in_maps:
            m.pop("gamma_r")
            m.pop("beta_r")
    res = run_bass_kernel_spmd(nc, in_maps, list(range(NCORES)))
    full = np.empty((B, S, D), np.float32)
    for c in range(NCORES):
        b, r = c // 4, c % 4
        o = res.results[c]["out"]
        for j in range(NQB):
            full[b, QB * j + 128 * r:QB * j + 128 * r + 128, :] = \
                o[128 * j:128 * j + 128]
    return full
